# revision 22
# baseline (speedup 1.0000x reference)
"""DecorrelateBN Trainium2 kernel (8-core SPMD, raw Bass).

Math (matches reference):
  x0 = x * mask  (mask zeroes dropped points); groups g = channels [8g, 8g+8)
  Gram65 = sum over (t, j) of [x0_j; mask]^T [x0_j; mask]  (X_j[t,g] = x0[t,8g+j])
  -> all-reduced over 8 cores.  s = Gram65[64,:64], Gn = Gram65[64,64]
  cov = (Gram0 - s (s/Gn)^T)/Gn + eps I ; dcv = NewtonSchulz_isqrt(cov, 5)
  out[t, 8g'+j] = w[8g'+j]*(sum_g dcv[g,g'] x0[t,8g+j] - mask[t]*dm[g']) + b
  where dm = dcv @ (s/Gn).

Single pass over x (32MB/core): phase 1 streams x, casts masked bf16 tiles,
accumulates Gram65 in PSUM AND transposes each tile into a SBUF-resident
bf16 x^T cache (16MB).  Phase 2: AllReduce (17KB) + Newton-Schulz on-device,
then builds the 8 per-j [64,64] rhs blocks (dcv * w, bf16) + the [2,512]
(-w*dm ; bias) row pair.  Phase 3 applies the block-diagonal deconv as 8
small matmuls per 128-point tile (exploiting the kron(dcv, I8) sparsity)
plus one K=2 matmul adding mask*(-w*dm) and bias, then evacuates
PSUM->SBUF with the (j,g)->(g,j) channel reorder and DMAs out.
"""
import sys

sys.path.insert(0, "/opt/trn_rl_repo")

import numpy as np
import ml_dtypes
import concourse.bass as bass
from concourse import mybir
from concourse.bass_utils import run_bass_kernel_spmd

F32 = mybir.dt.float32
BF16 = mybir.dt.bfloat16

N_CORES = 8
B, N, C = 32, 4096, 512
G = 64
J = C // G              # 8
PTS = (B // N_CORES) * N            # 16384 points per core
NST = PTS // 512                    # 32 supertiles
NT = PTS // 128                     # 128 tiles
EPS = 1e-4
N_ITER = 5

_cache = {}


class _SemOffset:
    """Engine proxy that offsets wait_ge values by a per-iteration base so
    the kernel body can be emitted `reps` times for slope timing."""

    def __init__(self, eng, base):
        self._e = eng
        self._b = base

    def wait_ge(self, sem, v):
        return self._e.wait_ge(sem, v + self._b.get(id(sem), 0))

    def __getattr__(self, n):
        return getattr(self._e, n)


def build(n_cores, use_collective=True, reps=1, do_p3=True):
    nc = bass.Bass(target_bir_lowering=False)

    x = nc.declare_dram_parameter("x", [PTS, C], F32, isOutput=False)
    maskt = nc.declare_dram_parameter("maskt", [128, NT], F32, isOutput=False)
    mrow2 = nc.declare_dram_parameter("mrow2", [2, NT, 128], BF16, isOutput=False)
    eyestk = nc.declare_dram_parameter("eyestk", [64, 128], F32, isOutput=False)
    wmaskp = nc.declare_dram_parameter("wmaskp", [128, 4, 128], F32, isOutput=False)
    wjm = nc.declare_dram_parameter("wjm", [1, 512], F32, isOutput=False)
    biasjm = nc.declare_dram_parameter("biasjm", [1, 512], BF16, isOutput=False)
    eyebf = nc.declare_dram_parameter("eyebf", [128, 128], BF16, isOutput=False)
    eye64 = nc.declare_dram_parameter("eye64", [64, 64], F32, isOutput=False)
    epseye = nc.declare_dram_parameter("epseye", [64, 64], F32, isOutput=False)
    i15 = nc.declare_dram_parameter("i15", [64, 64], F32, isOutput=False)
    ones8 = nc.declare_dram_parameter("ones8", [128, 8], F32, isOutput=False)
    onesbf = nc.declare_dram_parameter("onesbf", [128, 1], BF16, isOutput=False)
    ones64 = nc.declare_dram_parameter("ones64", [64, 1], F32, isOutput=False)
    ones65 = nc.declare_dram_parameter("ones65", [65, 64], F32, isOutput=False)
    out = nc.declare_dram_parameter("out", [PTS, C], F32, isOutput=True)

    cc_in = nc.dram_tensor("cc_in", [65, 65], F32)
    cc_out = nc.dram_tensor("cc_out", [65, 65], F32, addr_space="Shared")

    CONSTS = [("maskt_s", maskt, [128, NT], F32),
              ("eyestk_s", eyestk, [64, 128], F32),
              ("wmaskp_s", wmaskp, [128, 4, 128], F32),
              ("wjm_s", wjm, [1, 512], F32),
              ("eyebf_s", eyebf, [128, 128], BF16),
              ("eye64_s", eye64, [64, 64], F32),
              ("epseye_s", epseye, [64, 64], F32),
              ("i15_s", i15, [64, 64], F32),
              ("ones8_s", ones8, [128, 8], F32),
              ("onesbf_s", onesbf, [128, 1], BF16),
              ("ones64_s", ones64, [64, 1], F32),
              ("ones65_s", ones65, [65, 64], F32),
              ("mrow2_s", mrow2, [2, NT, 128], BF16)]
    NCONST = len(CONSTS) + 1  # + biasjm -> nb2 row 1

    import contextlib
    ctx = contextlib.ExitStack()
    sb = {}
    for nm, _, shp, dt in CONSTS:
        sb[nm] = ctx.enter_context(nc.sbuf_tensor(nm, shp, dt))

    xst = [ctx.enter_context(nc.sbuf_tensor(f"xst{i}", [128, 2, 512], F32))
           for i in range(3)]
    xm = [ctx.enter_context(nc.sbuf_tensor(f"xm{i}", [128, 520], BF16))
          for i in range(4)]
    xt = ctx.enter_context(nc.sbuf_tensor("xt", [128, NT, 4, 128], BF16))
    osb = [ctx.enter_context(nc.sbuf_tensor(f"osb{i}", [128, 4, 512], F32))
           for i in range(2)]

    gram_sb = ctx.enter_context(nc.sbuf_tensor("gram_sb", [65, 65], F32))
    g2 = ctx.enter_context(nc.sbuf_tensor("g2", [65, 66], F32))
    # phase-2 small tiles
    lane64 = ctx.enter_context(nc.sbuf_tensor("lane64", [65, 66], F32))
    covu = ctx.enter_context(nc.sbuf_tensor("covu", [64, 64], F32))
    cov = ctx.enter_context(nc.sbuf_tensor("cov", [64, 64], F32))
    scr64 = ctx.enter_context(nc.sbuf_tensor("scr64", [64, 64], F32))
    rowsq = ctx.enter_context(nc.sbuf_tensor("rowsq", [64, 1], F32))
    smalls = ctx.enter_context(nc.sbuf_tensor("smalls", [64, 8], F32))
    # smalls cols: 0 invcol, 1 ninvcol, 2 rcol, 3 mcol; row0 of col 4: nfro,
    # 5: ninv, 6: sq2, 7: rinv
    YZ = ctx.enter_context(nc.sbuf_tensor("YZ", [64, 128], F32))
    Tt = ctx.enter_context(nc.sbuf_tensor("Tt", [64, 64], F32))
    dcv = ctx.enter_context(nc.sbuf_tensor("dcv", [64, 64], F32))
    rhs8p = ctx.enter_context(nc.sbuf_tensor("rhs8p", [128, 4, 128], BF16))
    nb2 = ctx.enter_context(nc.sbuf_tensor("nb2", [2, 512], BF16))

    ps6 = ctx.enter_context(nc.psum_tensor("ps6", [128, 512], F32))
    ps7 = ctx.enter_context(nc.psum_tensor("ps7", [128, 512], F32))
    pst = [ctx.enter_context(nc.psum_tensor(f"pst{i}", [128, 1024], BF16))
           for i in range(3)]
    pap = [ctx.enter_context(nc.psum_tensor(f"pap{i}", [128, 512], F32))
           for i in range(3)]

    sems = {}
    for nm in ["sconst", "sx0", "sx1", "sx2", "sxm", "sgr", "stp", "sevt",
               "sge", "scol", "sccd", "sp2", "sap", "sev2a", "sev2v",
               "sod0", "sod1"]:
        sems[nm] = ctx.enter_context(nc.semaphore(nm))
    (sconst, sx0, sx1, sx2, sxm, sgr, stp, sevt, sge, scol, sccd, sp2, sap,
     sev2a, sev2v, sod0, sod1) = (
        sems[k] for k in ["sconst", "sx0", "sx1", "sx2", "sxm", "sgr", "stp",
                          "sevt", "sge", "scol", "sccd", "sp2", "sap",
                          "sev2a", "sev2v", "sod0", "sod1"])
    sx = [sx0, sx1, sx2]
    sod = [sod0, sod1]

    x_r = x[:].rearrange("(l k p) c -> l p k c", p=128, k=2)
    out_r = out[:].rearrange("(s k p) c -> s p k c", p=128, k=4)
    NL = NT // 2

    # ---------------- phase 2 step list ----------------
    A = mybir.AluOpType
    ACT_F = mybir.ActivationFunctionType
    p2_steps = []

    def step(eng):
        def deco(fn):
            p2_steps.append((eng, fn))
            return fn
        return deco

    invgn = lane64[64:65, 0:1]
    meanrow = lane64[64:65, 1:65]
    invcol = smalls[0:64, 0:1]
    ninvcol = smalls[0:64, 1:2]
    rcol = smalls[0:64, 2:3]
    mcol = smalls[0:64, 3:4]
    nfro = smalls[0:1, 4:5]
    ninv = smalls[0:1, 5:6]
    sq2 = smalls[0:1, 6:7]
    rinv = smalls[0:1, 7:8]

    step("vector")(lambda e: e.reciprocal(invgn, g2[64:65, 64:65]))
    step("vector")(lambda e: e.tensor_scalar_mul(meanrow, g2[64:65, 0:64], invgn))
    step("tensor")(lambda e: e.matmul(ps7[0:64, 0:64], g2[64:65, 0:64],
                                      meanrow, start=True, stop=True))
    step("tensor")(lambda e: e.matmul(ps7[0:64, 64:65], sb["ones65_s"][64:65, :],
                                      invgn, start=True, stop=True))
    step("vector")(lambda e: e.tensor_copy(invcol, ps7[0:64, 64:65]))
    step("vector")(lambda e: e.tensor_tensor(covu[:], g2[0:64, 0:64],
                                             ps7[0:64, 0:64], A.subtract))
    step("vector")(lambda e: e.scalar_tensor_tensor(
        out=cov[:], in0=covu[:], scalar=invcol, in1=sb["epseye_s"][:],
        op0=A.mult, op1=A.add))
    step("vector")(lambda e: e.tensor_tensor(scr64[:], cov[:], cov[:], A.mult))
    step("vector")(lambda e: e.tensor_reduce(rowsq[:], scr64[:],
                                             mybir.AxisListType.X, A.add))
    step("tensor")(lambda e: e.matmul(ps7[0:1, 65:66], rowsq[:],
                                      sb["ones64_s"][:], start=True, stop=True))
    step("scalar")(lambda e: e.activation(nfro, ps7[0:1, 65:66], ACT_F.Sqrt))
    step("vector")(lambda e: e.reciprocal(ninv, nfro))
    step("tensor")(lambda e: e.matmul(ps7[0:64, 66:67], sb["ones65_s"][0:1, :],
                                      ninv, start=True, stop=True))
    step("vector")(lambda e: e.tensor_copy(ninvcol, ps7[0:64, 66:67]))
    step("vector")(lambda e: e.tensor_scalar_mul(YZ[:, 0:64], cov[:],
                                                 ninvcol))
    # iter 0: Z=I so T = 1.5I - 0.5 Y ; Y1 = Y T ; Z1 = T
    step("vector")(lambda e: e.scalar_tensor_tensor(
        out=Tt[:], in0=YZ[:, 0:64], scalar=-0.5, in1=sb["i15_s"][:],
        op0=A.mult, op1=A.add))
    step("tensor")(lambda e: e.matmul(ps7[0:64, 0:64], YZ[:, 0:64], Tt[:],
                                      start=True, stop=True))
    step("vector")(lambda e: e.tensor_copy(YZ[:, 0:64], ps7[0:64, 0:64]))
    step("vector")(lambda e: e.tensor_copy(YZ[:, 64:128], Tt[:]))
    for _ in range(N_ITER - 1):
        step("tensor")(lambda e: e.matmul(ps6[0:64, 0:64], YZ[:, 64:128],
                                          YZ[:, 0:64], start=True, stop=True))
        step("vector")(lambda e: e.scalar_tensor_tensor(
            out=Tt[:], in0=ps6[0:64, 0:64], scalar=-0.5, in1=sb["i15_s"][:],
            op0=A.mult, op1=A.add))
        step("tensor")(lambda e: e.matmul(ps7[0:64, 0:64], YZ[:, 0:64], Tt[:],
                                          start=True, stop=True))
        step("tensor")(lambda e: e.matmul(ps7[0:64, 64:128], Tt[:],
                                          YZ[:, 64:128], start=True,
                                          stop=True))
        step("vector")(lambda e: e.tensor_copy(YZ[:], ps7[0:64, 0:128]))
    step("scalar")(lambda e: e.activation(sq2, nfro, ACT_F.Sqrt))
    step("vector")(lambda e: e.reciprocal(rinv, sq2))
    step("tensor")(lambda e: e.matmul(ps6[0:64, 67:68], sb["ones65_s"][0:1, :],
                                      rinv, start=True, stop=True))
    step("vector")(lambda e: e.tensor_copy(rcol, ps6[0:64, 67:68]))
    step("vector")(lambda e: e.tensor_scalar_mul(dcv[:], YZ[:, 64:128], rcol))
    step("vector")(lambda e: e.tensor_tensor(mcol, g2[0:64, 65:66],
                                             invcol, A.mult))
    step("tensor")(lambda e: e.matmul(ps7[0:1, 128:192], mcol, dcv[:],
                                      start=True, stop=True))
    step("tensor")(lambda e: e.matmul(ps7[0:128, 192:256], sb["eyestk_s"][:],
                                      dcv[:], start=True, stop=True))
    step("tensor")(lambda e: e.matmul(ps7[0:128, 256:320], sb["eyestk_s"][:],
                                      dcv[:], start=True, stop=True))
    for a in range(4):
        step("vector")(lambda e, a=a: e.tensor_tensor(
            rhs8p[:, a, :], ps7[0:128, 192:320], sb["wmaskp_s"][:, a, :],
            A.mult))
    for j in range(8):
        step("vector")(lambda e, j=j: e.scalar_tensor_tensor(
            out=nb2[0:1, 64 * j:64 * j + 64], in0=ps7[0:1, 128:192],
            scalar=-1.0, in1=sb["wjm_s"][0:1, 64 * j:64 * j + 64],
            op0=A.mult, op1=A.mult))
    NSDONE = len(p2_steps)

    def emit_p2(eng_name, eng):
        for t, (enm, fn) in enumerate(p2_steps):
            if enm != eng_name:
                continue
            if t == 0:
                eng.wait_ge(scol, 48)
            else:
                eng.wait_ge(sp2, t)
            fn(eng).then_inc(sp2, 1)

    TOT = {id(sx0): 16 * 22, id(sx1): 16 * 21, id(sx2): 16 * 21,
           id(sxm): NT,
           id(sgr): 1, id(stp): NT, id(sevt): NT, id(sge): 1, id(scol): 48,
           id(sccd): 1, id(sp2): NSDONE, id(sap): NT,
           id(sev2a): NT // 2, id(sev2v): NT // 2,
           id(sod0): 16 * NST // 2, id(sod1): 16 * NST // 2}

    def base_for(it):
        return {k: it * v for k, v in TOT.items()}

    def _emit_sync_body(sync, it):
        if it > 0 and do_p3:
            sync.wait_ge(sod0, 0)  # all prev-iteration stores done
            sync.wait_ge(sod1, 0)
        elif it > 0:
            sync.wait_ge(sp2, 0)  # prev-iteration phase 2 done
        for l in range(NL):
            if l >= 3:
                sync.wait_ge(sxm, 2 * (l - 2))
                sync.wait_ge(sx[l % 3], 16 * (l // 3))
            sync.dma_start(out=xst[l % 3][:],
                           in_=x_r[l]).then_inc(sx[l % 3], 16)

    def _emit_scalar_body(scalar):
        # ---- phase 1: masked bf16 casts ----
        for i in range(NT):
            s, k = i // 4, i % 4
            l = i // 2
            if i % 2 == 0:
                scalar.wait_ge(sx[l % 3], 16 * (l // 3 + 1))
            if i >= 4:
                scalar.wait_ge(stp, i - 3)
            mcolv = sb["maskt_s"][:, i:i + 1]
            scalar.activation(xm[i % 4][:, 0:512].rearrange(
                                  "p (j g) -> p j g", j=8),
                              xst[l % 3][:, i % 2, :].rearrange(
                                  "p (g j) -> p j g", j=8),
                              ACT_F.Copy, bias=0.0, scale=mcolv)
            scalar.activation(xm[i % 4][:, 512:513], sb["ones8_s"][:, 0:1],
                              ACT_F.Copy, bias=0.0,
                              scale=mcolv).then_inc(sxm, 1)
        # ---- phase 2 ----
        emit_p2("scalar", scalar)
        # ---- phase 3: half the output evacuations (pure reorder copy) ----
        for i in range(1, NT, 2) if do_p3 else []:
            s, k = i // 4, i % 4
            scalar.wait_ge(sap, i + 1)
            if s >= 2:
                scalar.wait_ge(sod[s % 2], 16 * (s // 2))
            scalar.copy(
                osb[s % 2][:, k, :].rearrange("p (g j) -> p g j", j=8),
                pap[i % 3][:, 0:512].rearrange("p (j g) -> p g j", j=8)
            ).then_inc(sev2a, 1)

    def _emit_tensor_body(tensor):
        # ---- phase 1: Gram (j-major) + masked sums + transposes ----
        for i in range(NT):
            tensor.wait_ge(sxm, i + 1)
            if i >= 3:
                tensor.wait_ge(sevt, i - 2)
            for j in range(8):
                tensor.matmul(ps6[0:64, 0:64], xm[i % 4][:, 64 * j:64 * j + 64],
                              xm[i % 4][:, 64 * j:64 * j + 64],
                              start=(i == 0 and j == 0), stop=False,
                              skip_group_check=True)
            tensor.matmul(ps6[64:65, 0:512], sb["onesbf_s"][:],
                          xm[i % 4][:, 0:512], start=(i == 0),
                          stop=(i == NT - 1), skip_group_check=True)
            mm = tensor.matmul(ps7[64:65, 0:1], sb["onesbf_s"][:],
                               xm[i % 4][:, 512:513], start=(i == 0),
                               stop=(i == NT - 1), skip_group_check=True)
            if i == NT - 1:
                mm.then_inc(sgr, 1)
            for a in range(4):
                mm = tensor.transpose(
                    pst[i % 3][:, a * 128:(a + 1) * 128],
                    xm[i % 4][:, a * 128:(a + 1) * 128],
                    sb["eyebf_s"][:])
            mm.then_inc(stp, 1)
        # ---- phase 2 ----
        emit_p2("tensor", tensor)
        # ---- phase 3: block-diag apply ----
        for i in range(NT) if do_p3 else []:
            if i == 0:
                tensor.wait_ge(sp2, NSDONE)
            tensor.wait_ge(sevt, i + 1)
            if i >= 3:
                tensor.wait_ge(sev2v if (i - 3) % 2 == 0 else sev2a,
                               (i - 3) // 2 + 1)
            for a in range(4):
                tensor.matmul(pap[i % 3][:, 128 * a:128 * a + 128],
                              xt[:, i, a, :],
                              rhs8p[:, a, :], start=(a == 0),
                              stop=False, skip_group_check=True)
            tensor.matmul(pap[i % 3][:, 0:512], sb["mrow2_s"][:, i, :],
                          nb2[:], start=False, stop=True,
                          skip_group_check=True).then_inc(sap, 1)

    def _emit_vector_body(vector):
        # ---- phase 1: x^T cache evacuation ----
        for i in range(NT):
            vector.wait_ge(stp, i + 1)
            vector.tensor_copy(
                xt[:, i, :, :],
                pst[i % 3][:, 0:512]).then_inc(sevt, 1)
        # gram evacuation
        vector.wait_ge(sgr, 1)
        vector.memset(gram_sb[0:64, 64:65], 0.0)
        vector.tensor_copy(gram_sb[0:64, 0:64], ps6[0:64, 0:64])
        vector.tensor_reduce(
            gram_sb[64:65, 0:64],
            ps6[64:65, 0:512].rearrange("p (j g) -> p g j", j=8),
            mybir.AxisListType.X, A.add)
        vector.tensor_scalar_mul(gram_sb[64:65, 64:65],
                                 ps7[64:65, 0:1], 8.0).then_inc(sge, 1)
        # ---- phase 2 ----
        emit_p2("vector", vector)
        # ---- phase 3: other half of output evacuations ----
        for i in range(0, NT, 2) if do_p3 else []:
            s, k = i // 4, i % 4
            vector.wait_ge(sap, i + 1)
            if s >= 2:
                vector.wait_ge(sod[s % 2], 16 * (s // 2))
            vector.tensor_copy(
                osb[s % 2][:, k, :].rearrange("p (g j) -> p g j", j=8),
                pap[i % 3][:, 0:512].rearrange("p (j g) -> p g j", j=8)
            ).then_inc(sev2v, 1)

    def _emit_gpsimd_body(gpsimd):
        gpsimd.wait_ge(sge, 1)
        gpsimd.dma_start(out=cc_in[:], in_=gram_sb[:]).then_inc(scol, 16)
        gpsimd.wait_ge(scol, 16)
        if use_collective:
            gpsimd.collective_compute(
                "AllReduce", mybir.AluOpType.add,
                replica_groups=[list(range(n_cores))],
                ins=[cc_in[:]],
                outs=[cc_out[:]]).then_inc(sccd, 1)
            gpsimd.wait_ge(sccd, 1)
            gpsimd.dma_start(out=g2[0:65, 0:65], in_=cc_out[:]).then_inc(scol, 16)
        else:
            gpsimd.dma_start(out=g2[0:65, 0:65], in_=cc_in[:]).then_inc(scol, 16)
        gpsimd.wait_ge(scol, 32)
        gpsimd.dma_start(out=g2[0:64, 65:66],
                         in_=g2[64:65, 0:64]).then_inc(scol, 16)
        # phase 3 stores
        for s in range(NST) if do_p3 else []:
            gpsimd.wait_ge(sev2a, 2 * s + 2)
            gpsimd.wait_ge(sev2v, 2 * s + 2)
            if s >= 2:
                gpsimd.wait_ge(sod[s % 2], 16 * (s // 2))
            gpsimd.dma_start(out=out_r[s],
                             in_=osb[s % 2][:]).then_inc(sod[s % 2], 16)
        if do_p3:
            gpsimd.wait_ge(sod0, 16 * NST // 2)
            gpsimd.wait_ge(sod1, 16 * NST // 2)
        else:
            gpsimd.wait_ge(sp2, NSDONE)

    with nc.Block() as block:

        @block.sync
        def _(sync0):
            for nm, src, shp, dt in CONSTS:
                sync0.dma_start(out=sb[nm][:], in_=src[:]).then_inc(sconst, 16)
            sync0.dma_start(out=nb2[1:2, :], in_=biasjm[:]).then_inc(sconst, 16)
            for it in range(reps):
                _emit_sync_body(_SemOffset(sync0, base_for(it)), it)

        @block.scalar
        def _(scalar0):
            scalar0.wait_ge(sconst, 16 * NCONST)
            for it in range(reps):
                _emit_scalar_body(_SemOffset(scalar0, base_for(it)))

        @block.tensor
        def _(tensor0):
            tensor0.wait_ge(sconst, 16 * NCONST)
            for it in range(reps):
                _emit_tensor_body(_SemOffset(tensor0, base_for(it)))

        @block.vector
        def _(vector0):
            vector0.wait_ge(sconst, 16 * NCONST)
            for it in range(reps):
                _emit_vector_body(_SemOffset(vector0, base_for(it)))

        @block.gpsimd
        def _(gpsimd0):
            for it in range(reps):
                _emit_gpsimd_body(_SemOffset(gpsimd0, base_for(it)))

    ctx.close()
    return nc


def _host_aux():
    w = _cache["weight"].astype(np.float32)
    b = _cache["bias"].astype(np.float32)
    w2 = w.reshape(64, 8)            # w2[gp, j] = w[8*gp + j]
    wjmr = np.ascontiguousarray(w2.T.reshape(1, 512))  # [0, 64j+gp]
    wmaskp = np.zeros((128, 4, 128), dtype=np.float32)
    for j2 in range(2):
        for a in range(4):
            wmaskp[64 * j2:64 * j2 + 64, a, 64 * j2:64 * j2 + 64] = \
                w2[:, 2 * a + j2][None, :]
    eyestk = np.concatenate([np.eye(64), np.eye(64)], axis=1).astype(np.float32)
    b2 = b.reshape(64, 8)
    biasjm = np.ascontiguousarray(b2.T.reshape(1, 512)).astype(ml_dtypes.bfloat16)
    aux = {
        "wmaskp": wmaskp,
        "eyestk": eyestk,
        "wjm": wjmr,
        "biasjm": biasjm,
        "eyebf": np.eye(128, dtype=ml_dtypes.bfloat16),
        "eye64": np.eye(64, dtype=np.float32),
        "epseye": (EPS * np.eye(64)).astype(np.float32),
        "i15": (1.5 * np.eye(64)).astype(np.float32),
        "ones8": np.ones((128, 8), dtype=np.float32),
        "onesbf": np.ones((128, 1), dtype=ml_dtypes.bfloat16),
        "ones64": np.ones((64, 1), dtype=np.float32),
        "ones65": np.ones((65, 64), dtype=np.float32),
    }
    return aux


def make_in_maps(x, mask, weight, bias):
    _cache["weight"] = np.asarray(weight, dtype=np.float32)
    _cache["bias"] = np.asarray(bias, dtype=np.float32)
    aux = _host_aux()
    x = np.asarray(x, dtype=np.float32)
    mask = np.asarray(mask)
    in_maps = []
    bpc = B // N_CORES
    for c in range(N_CORES):
        xc = np.ascontiguousarray(
            x[c * bpc:(c + 1) * bpc].reshape(PTS, C))
        m = mask[c * bpc:(c + 1) * bpc].reshape(PTS).astype(np.float32)
        maskt = np.ascontiguousarray(m.reshape(NT, 128).T)
        mrow2 = np.empty((2, NT, 128), dtype=ml_dtypes.bfloat16)
        mrow2[0] = m.reshape(NT, 128)
        mrow2[1] = 1.0
        im = {"x": xc, "maskt": maskt, "mrow2": mrow2}
        im.update(aux)
        in_maps.append(im)
    return in_maps


def kernel(coords, x, mask, weight, bias, _trace=False):
    in_maps = make_in_maps(x, mask, weight, bias)

    if "nc" not in _cache:
        _cache["nc"] = build(N_CORES)
    nc = _cache["nc"]

    res = run_bass_kernel_spmd(nc, in_maps, core_ids=list(range(N_CORES)),
                               trace=_trace)
    bpc = B // N_CORES
    outs = [res.results[c]["out"].reshape(bpc, N, C) for c in range(N_CORES)]
    full = np.concatenate(outs, axis=0)
    if _trace:
        return full, res
    return full
